# revision 40
# baseline (speedup 1.0000x reference)
"""Transformer decoder layer (pre-norm, self-attn + cross-attn + FFN) on 8
Trainium2 NeuronCores.

Sharding: core c handles batch b = c//2 and the contiguous half of the 1024
target tokens h = c%2 (512 query rows each). K/V work is duplicated within
each batch pair so there are no collectives; every core runs an identical
program on different data. The host rotates the token order per core so that
each core's own tokens are always columns [0, 512) -> one uniform SPMD
program.

On-device layout is feature-major ([d_model, token]) throughout, so no
on-device transposes are needed: the host pre-transposes x / memory / weights
and transposes the output back. Masks are converted to additive fp32 masks on
the host.

The heavy projections (QKVO of both attentions + both FFN layers) run as
fp8e4m3 DoubleRow matmuls (2x PE rate): weights are scaled by 2048 and
activations by 32 on their way into fp8; the product scale 2^16 is divided
back out in the bias-add / exp-scale stage. Attention scores and AV run in
bf16 (DoubleRow has no advantage there and softmax smooths Q/K quantization
noise, while V/O errors would hit the residual directly).
"""

import numpy as np
import ml_dtypes
from contextlib import ExitStack

import concourse.bass as bass
import concourse.bacc as bacc
import concourse.tile as tile
from concourse import mybir
from concourse.bass_utils import run_bass_kernel_spmd

D = 1024        # d_model
H = 16          # heads
DK = 64         # head dim
DFF = 4096
B = 4
T = 1024        # tgt/src len
OWN = 512       # query rows per core
P = 128         # partitions
NKC = D // P    # 8 feature chunks
NPC = NKC // 2  # 4 feature pair-chunks (fp8 DoubleRow)
NSC = T // P    # 8 s-chunks
NFC = DFF // P  # 32 ffn chunks
NFP = NFC // 2  # 16 ffn pair-chunks
EPS = 1e-6

F32 = mybir.dt.float32
F32R = mybir.dt.float32r
BF16 = mybir.dt.bfloat16
FP8 = mybir.dt.float8e4
NPBF = ml_dtypes.bfloat16
NPF8 = ml_dtypes.float8_e4m3
AF = mybir.ActivationFunctionType
ALU = mybir.AluOpType
DR = mybir.MatmulPerfMode.DoubleRow

# which projection groups run as fp8 DoubleRow
CFG = {"sa_qkv": True, "ca_qkv": True, "sa_o": False, "ca_o": False,
       "ff1": False, "ff2": False, "ff_f32r": False}

SW = 2048.0     # fp8 weight scale
SA = 32.0       # fp8 activation scale
MASKV = float(2.0 ** 62)   # pre-exp-scale additive mask magnitude


def _gains(cfg):
    """Per-group scale constants shared by host prep and program build."""
    g = {}
    g["h1"] = SA if cfg["sa_qkv"] else 1.0
    g["h2"] = SA if cfg["ca_qkv"] else 1.0
    g["h3"] = SA if cfg["ff1"] else 1.0
    g["mem"] = SA if cfg["ca_qkv"] else 1.0
    g["aT"] = SA if cfg["ff2"] else 1.0
    g["ot_sa"] = SA if cfg["sa_o"] else 1.0
    g["ot_ca"] = SA if cfg["ca_o"] else 1.0
    g["w_sa_qkv"] = SW if cfg["sa_qkv"] else 1.0
    g["w_ca_qkv"] = SW if cfg["ca_qkv"] else 1.0
    g["w_sa_o"] = SW if cfg["sa_o"] else 1.0
    g["w_ca_o"] = SW if cfg["ca_o"] else 1.0
    g["w_ff1"] = SW if cfg["ff1"] else 1.0
    g["w_ff2"] = SW if cfg["ff2"] else 1.0
    # derived
    g["qk_sa"] = g["w_sa_qkv"] * g["h1"]         # gain on sa q and k
    g["qk_ca"] = g["w_ca_qkv"] * 1.0             # ca k gain uses mem scale
    g["q_ca"] = g["w_ca_qkv"] * g["h2"]
    g["k_ca"] = g["w_ca_qkv"] * g["mem"]
    g["v_sa"] = g["w_sa_qkv"] * g["h1"]
    g["v_ca"] = g["w_ca_qkv"] * g["mem"]
    g["att_sa"] = 1.0 / (8.0 * g["qk_sa"] * g["qk_sa"])
    g["att_ca"] = 1.0 / (8.0 * g["q_ca"] * g["k_ca"])
    g["prb_sa"] = g["ot_sa"] / g["v_sa"]
    g["prb_ca"] = g["ot_ca"] / g["v_ca"]
    g["odeq_sa"] = 1.0 / (g["w_sa_o"] * g["ot_sa"])
    g["odeq_ca"] = 1.0 / (g["w_ca_o"] * g["ot_ca"])
    g["relu_s"] = g["aT"] / (g["w_ff1"] * g["h3"])
    g["ydeq"] = 1.0 / (g["w_ff2"] * g["aT"])
    return g


# ---------------------------------------------------------------------------
# program builder (identical for every core; only DRAM contents differ)
# ---------------------------------------------------------------------------

def build_program(repeat=1, mask_mode="split", cfg=None, consts=None):
    """consts: dict of numpy arrays for weight-class tensors. When given,
    they are embedded in the NEFF (kind=Const) and never re-staged per
    execution — only x / memory / the small dynamic mask columns remain
    runtime inputs. When None, everything is an ExternalInput (legacy)."""
    cfg = dict(CFG if cfg is None else cfg)
    g = _gains(cfg)
    h1_dt = FP8 if cfg["sa_qkv"] else BF16
    h2_dt = FP8 if cfg["ca_qkv"] else BF16
    ff_fb = F32R if cfg.get("ff_f32r") else BF16
    h3_dt = FP8 if cfg["ff1"] else ff_fb
    mem_dt = FP8 if cfg["ca_qkv"] else BF16
    aT_dt = FP8 if cfg["ff2"] else ff_fb
    ot_sa_dt = FP8 if cfg["sa_o"] else BF16
    ot_ca_dt = FP8 if cfg["ca_o"] else BF16
    wqkv_sa_dt = FP8 if cfg["sa_qkv"] else BF16
    wqkv_ca_dt = FP8 if cfg["ca_qkv"] else BF16
    wo_sa_dt = FP8 if cfg["sa_o"] else BF16
    wo_ca_dt = FP8 if cfg["ca_o"] else BF16
    w1_dt = FP8 if cfg["ff1"] else ff_fb
    w2_dt = FP8 if cfg["ff2"] else ff_fb

    nc = bacc.Bacc(None)
    dr = {}

    def din(name, shape, dt=F32):
        if consts is not None and name in consts:
            arr = consts[name]
            assert list(arr.shape) == list(shape), (name, arr.shape, shape)
            dr[name] = nc.inline_tensor(arr, name=name)
        else:
            dr[name] = nc.dram_tensor(name, list(shape), dt,
                                      kind="ExternalInput")
        return dr[name]

    din("ones_r", [P, P], F32R)
    din("trimask", [P, P])                 # lower-tri additive mask block
    din("xT", [D, T], BF16)                # batch-b x, transposed, own first
    din("memT", [NPC, P, 2, T], mem_dt)    # memory[b] transposed, pairs
    if mask_mode != "causal":
        din("maskT", [T, OWN])             # additive tgt mask (pre-scale)
    din("lnwrows", [1, 3 * D], F32R)       # LN w rows, concat (PE bcast)
    # static small per-partition tensors packed into one tile:
    # cols: [sa_bq 8][sa_bk 8][sa_bo 8][ca_bq 8][ca_bk 8][ca_bo 8]
    #       [b1 32][b2 8][ln1_b 8][ln2_b 8][ln3_b 8]
    din("smalls", [P, 112])
    # per-core mask-derived columns: [smask 8][tailb 4]
    din("smdyn", [P, 12])
    for pre, wdt, odt in (("sa", wqkv_sa_dt, wo_sa_dt),
                          ("ca", wqkv_ca_dt, wo_ca_dt)):
        din(f"{pre}_wq", [NPC, P, 2, D], wdt)
        din(f"{pre}_wk", [NPC, P, 2, D], wdt)
        din(f"{pre}_wv", [NPC, P, 2, D], wdt)
        din(f"{pre}_wo", [NPC, P, 2, D], odt)
        din(f"{pre}_bv", [D], F32R)      # placed at partition 64
    din("w1", [NPC, P, 2, DFF], w1_dt)
    din("w2", [NFP, P, 2, D], w2_dt)

    outT = nc.dram_tensor("outT", [D, OWN], F32, kind="ExternalOutput")

    def pair_mm(ps, w_tile, w_cols, src_tile, src_cols, pc, npc, pm,
                swap=False):
        """One contraction pair-chunk of a projection: DoubleRow when pm,
        else two plain matmuls over the j sub-slots."""
        st = (pc == 0)
        sp = (pc == npc - 1)
        if pm is not None:
            lhs = w_tile[:, :, w_cols] if not swap else src_tile[:, :, src_cols]
            rhs = src_tile[:, :, src_cols] if not swap else w_tile[:, :, w_cols]
            nc.tensor.matmul(ps, lhs, rhs, start=st, stop=sp, perf_mode=pm)
        else:
            for j in range(2):
                lhs = (w_tile[:, j, w_cols] if not swap
                       else src_tile[:, j, src_cols])
                rhs = (src_tile[:, j, src_cols] if not swap
                       else w_tile[:, j, w_cols])
                nc.tensor.matmul(ps, lhs, rhs,
                                 start=(st and j == 0), stop=(sp and j == 1))

    with ExitStack() as ctx:
        tc = ctx.enter_context(tile.TileContext(nc))
        ctx.enter_context(nc.allow_low_precision(
            reason="fp8/f32r staging for full-rate PE matmuls"))
        persist = ctx.enter_context(tc.tile_pool(name="persist", bufs=1))

        ones = persist.tile([P, P], F32R, tag="ones", name="ones")
        nc.gpsimd.dma_start(ones[:], dr["ones_r"][:])
        tri = persist.tile([P, P], F32, tag="tri", name="tri")
        nc.gpsimd.dma_start(tri[:], dr["trimask"][:])
        ones_f = persist.tile([P, 1], F32, tag="ones_f", name="ones_f")
        nc.vector.memset(ones_f[:], 1.0)
        ones_b = persist.tile([P, 1], BF16, tag="ones_b", name="ones_b")
        nc.vector.memset(ones_b[:], 1.0)
        eps_11 = persist.tile([1, 1], F32, tag="eps11", name="eps11")
        nc.vector.memset(eps_11[:], EPS)
        # constant rows used as lhsT of the attention renorm broadcast;
        # value folds the OT output scale and V dequant into the reciprocal
        prow = {}
        for key in ("prb_sa", "prb_ca"):
            val = g[key]
            if val not in prow:
                t = persist.tile([P, DK], F32, tag=f"pr{len(prow)}",
                                 name=f"pr{len(prow)}")
                nc.vector.memset(t[:], val)
                prow[val] = t.bitcast(F32R)
        prb_row = {"sa": prow[g["prb_sa"]], "ca": prow[g["prb_ca"]]}

        smalls = persist.tile([P, 112], F32, tag="smalls", name="smalls")
        nc.gpsimd.dma_start(smalls[:], dr["smalls"][:])
        smdyn = persist.tile([P, 12], F32, tag="smdyn", name="smdyn")
        nc.gpsimd.dma_start(smdyn[:], dr["smdyn"][:])
        _off = [0]

        def s_col(n):
            t = smalls[:, _off[0]:_off[0] + n]
            _off[0] += n
            return t

        bias = {}
        for pre in ("sa", "ca"):
            for nm in ("bq", "bk", "bo"):
                bias[f"{pre}_{nm}"] = s_col(NKC)
        bias["b1"] = s_col(NFC)
        bias["b2"] = s_col(NKC)
        lnp = {}
        for ln in ("ln1", "ln2", "ln3"):
            lnp[f"{ln}_b"] = s_col(NKC)
        smask = smdyn[:, 0:NSC]
        tailb = smdyn[:, NSC:NSC + 4]
        for pre in ("sa", "ca"):
            bv = persist.tile([P, D], F32R, tag=f"{pre}_bv", name=f"{pre}_bv")
            nc.gpsimd.dma_start(bv[64:65, :], dr[f"{pre}_bv"][None, :])
            bias[f"{pre}_bv"] = bv
        lnwr = persist.tile([1, 3 * D], F32R, tag="lnwrows", name="lnwrows")
        nc.gpsimd.dma_start(lnwr[:], dr["lnwrows"][:])
        lnrow = {"ln1": lnwr[0:1, 0:D], "ln2": lnwr[0:1, D:2 * D],
                 "ln3": lnwr[0:1, 2 * D:3 * D]}

        # residual (own tokens), lives to the end
        xcur = [persist.tile([P, OWN], F32, tag=f"xc{i}", name=f"xc{i}")
                for i in range(NKC)]

        tmp = ctx.enter_context(tc.tile_pool(name="tmp", bufs=2))
        stats = ctx.enter_context(tc.tile_pool(name="stats", bufs=1))

        # ------------------------------------------------------------------
        def layer_norm(nblk, src_get, dst_blocks, wrow, b_pn, ps_st):
            """Feature-major LN, h = (x - mean) * (w * rstd) + b.
            src_get(blk, kc) returns a [P, 512] AP; it may stream a fresh
            tile per call (it is called twice per chunk). w is folded into
            the PE broadcast (outer product w (x) rstd): 2-pass DVE
            normalize. wrow / b_pn carry the destination activation scale."""
            for blk in range(nblk):
                db = dst_blocks[blk]
                sb0 = src_get(blk, 0)
                ones_s = {F32R: ones[:, 0:1], F32: ones_f[:],
                          BF16: ones_b[:]}[sb0.dtype]
                ps_s = ps_st.tile([1, 512], F32, tag="lns", name="lns")
                ps_q = ps_st.tile([1, 512], F32, tag="lnq", name="lnq")
                for kc in range(NKC):
                    sbk = sb0 if kc == 0 else src_get(blk, kc)
                    nc.tensor.matmul(ps_s[:], ones_s, sbk,
                                     start=(kc == 0), stop=(kc == NKC - 1))
                    sq = tmp.tile([P, 512], F32R, tag="lnsq", name="lnsq", bufs=1)
                    nc.scalar.activation(sq[:], sbk, AF.Square)
                    nc.tensor.matmul(ps_q[:], ones[:, 0:1], sq[:],
                                     start=(kc == 0), stop=(kc == NKC - 1))
                s2 = stats.tile([1, 512], F32, tag="lnstA", name="lnstA")
                # s2 = (sum/sqrt(D*(D-1)))^2 = sum^2/(D*(D-1))
                nc.scalar.activation(s2[:], ps_s[:], AF.Square,
                                     scale=float(1.0 / np.sqrt(D * (D - 1.0))))
                var = stats.tile([1, 512], F32, tag="lnstB", name="lnstB")
                nc.vector.scalar_tensor_tensor(
                    var[:], ps_q[:], 1.0 / (D - 1.0), s2[:],
                    op0=ALU.mult, op1=ALU.subtract)
                std = stats.tile([1, 512], F32, tag="lnstA", name="lnstA")
                nc.scalar.activation(std[:], var[:], AF.Sqrt)
                nc.scalar.add(std[:], std[:], eps_11[:])
                rstd = stats.tile([1, 512], F32R, tag="lnstC",
                                  name="lnstC")
                nc.vector.reciprocal(rstd[:], std[:])
                mr = stats.tile([1, 512], F32R, tag="lnstB", name="lnstB")
                nc.vector.scalar_tensor_tensor(
                    mr[:], ps_s[:], 1.0 / D, rstd[:],
                    op0=ALU.mult, op1=ALU.mult)
                for kc in range(NKC):
                    wl = wrow[0:1, kc * P:(kc + 1) * P]
                    ps_rb = ps_st.tile([P, 512], F32, tag="ln_rb",
                                       name="ln_rb", bufs=1)
                    nc.tensor.matmul(ps_rb[:], wl, rstd[:],
                                     start=True, stop=True)
                    ps_mb = ps_st.tile([P, 512], F32, tag="ln_mb",
                                       name="ln_mb", bufs=1)
                    nc.tensor.matmul(ps_mb[:], wl, mr[:],
                                     start=True, stop=True)
                    t = tmp.tile([P, 512], F32, tag="lnt", name="lnt")
                    nc.vector.tensor_mul(t[:], src_get(blk, kc), ps_rb[:])
                    nc.vector.scalar_tensor_tensor(
                        db[kc], t[:], b_pn[:, kc:kc + 1], ps_mb[:],
                        op0=ALU.add, op1=ALU.subtract)

        # ------------------------------------------------------------------
        def load_w_pairs(wpool, wname, n=NPC, cols=D, dt=FP8):
            tiles = []
            for pc in range(n):
                wt = wpool.tile([P, 2, cols], dt, tag="w", name="w")
                nc.sync.dma_start(wt[:], dr[wname][pc])
                tiles.append(wt)
            return tiles

        def proj_fm(wpool, wname, bias_pn, src_blocks, dst, ps_acc,
                    w_tiles=None, w_dt=FP8, pm=DR):
            """dst[c][:, blk*512:..] = sum_pc W^T[pc,c].T @ src[blk][pc] + b.
            src_blocks[blk] is a list of NPC [P, 2, 512] pair tiles."""
            nblk = len(src_blocks)
            if w_tiles is None:
                w_tiles = load_w_pairs(wpool, wname, dt=w_dt)
            for c in range(NKC):
                pss = [ps_acc.tile([P, 512], F32, tag=f"proj{blk}",
                                   name=f"proj{blk}") for blk in range(nblk)]
                for pc in range(NPC):
                    for blk in range(nblk):
                        pair_mm(pss[blk][:], w_tiles[pc],
                                slice(c * P, (c + 1) * P),
                                src_blocks[blk][pc], slice(None), pc, NPC, pm)
                for blk in range(nblk):
                    nc.vector.tensor_scalar_add(
                        dst[c][:, blk * 512:(blk + 1) * 512],
                        pss[blk][:], bias_pn[:, c:c + 1])

        def proj_tm_vaug(wpool, wname, src_blocks, vaug, ps_acc, w_dt=FP8,
                         pm=DR):
            """Token-major V projection into [P, H, DK+1] aug tiles."""
            wv = load_w_pairs(wpool, wname, dt=w_dt)
            for dc in range(2):
                for st in range(NSC):
                    sb = src_blocks[st // 4]
                    t0 = (st % 4) * P
                    ps = ps_acc.tile([P, 512], F32, tag="proj0", name="proj0")
                    for pc in range(NPC):
                        pair_mm(ps[:], wv[pc],
                                slice(dc * 512, (dc + 1) * 512),
                                sb[pc], slice(t0, t0 + P), pc, NPC, pm,
                                swap=True)
                    nc.vector.tensor_copy(
                        vaug[st][:, 8 * dc:8 * (dc + 1), 0:DK],
                        ps[:].rearrange("p (h d) -> p h d", h=8))

        # ------------------------------------------------------------------
        def attention(KT, QT, vaug, OT, mask_tiles, tail_pn, smask_pn,
                      bv_tile, att_pools, s_att, prow_t, ot_dt,
                      causal=False, act_copy=False):
            ps_sc, ps_av, ps_rb, epool = att_pools
            nmask = 4 if causal else (
                len(mask_tiles) if mask_tiles is not None else 0)
            for c in range(NKC):
                po = [ps_av.tile([DK + 1, 512], F32, tag="po0", name="po0"),
                      ps_av.tile([DK + 1, 512], F32, tag="po1", name="po1")]
                for i in range(NSC):
                    # causal: masked chunk i only has live q-cols [128*i, 512)
                    off = 128 * i if (causal and i < nmask) else 0
                    pssc = [ps_sc.tile([P, 512], F32, tag="sc", name="sc"),
                            ps_sc.tile([P, 512], F32, tag="sc", name="sc")]
                    for h01 in (0, 1):
                        sl = slice(64 * h01, 64 * h01 + 64)
                        nc.tensor.matmul(
                            pssc[h01][:, off:], KT[c][sl, i * P:(i + 1) * P],
                            QT[c][sl, off:], start=True, stop=True)
                    for h01 in (0, 1):
                        e = epool.tile([P, 512], BF16, tag="e", name="e")
                        if causal and i < nmask:
                            # only the diagonal 128-col block is partially
                            # masked (lower-tri); the rest of the strip is
                            # fully visible
                            tm = epool.tile([P, P], F32, tag="emask",
                                            name="emask", bufs=2)
                            nc.vector.tensor_add(tm[:],
                                                 pssc[h01][:, off:off + P],
                                                 tri[:])
                            nc.scalar.activation(e[:, off:off + P], tm[:],
                                                 AF.Exp, scale=s_att)
                            if off + P < 512:
                                nc.scalar.activation(
                                    e[:, off + P:], pssc[h01][:, off + P:],
                                    AF.Exp, scale=s_att)
                        elif i < nmask:
                            tm = epool.tile([P, 512], F32, tag="emask",
                                            name="emask", bufs=2)
                            nc.vector.tensor_add(tm[:, off:],
                                                 pssc[h01][:, off:],
                                                 mask_tiles[i][:, off:])
                            nc.scalar.activation(e[:, off:], tm[:, off:],
                                                 AF.Exp, scale=s_att)
                        elif tail_pn is not None:
                            nc.scalar.activation(
                                e[:], pssc[h01][:], AF.Exp,
                                bias=tail_pn[:, i - 4:i - 3], scale=s_att)
                        elif smask_pn is not None:
                            nc.scalar.activation(e[:], pssc[h01][:], AF.Exp,
                                                 bias=smask_pn[:, i:i + 1],
                                                 scale=s_att)
                        else:
                            nc.scalar.activation(e[:], pssc[h01][:], AF.Exp,
                                                 scale=s_att)
                        nc.tensor.matmul(
                            po[h01][:, off:], vaug[i][:, 2 * c + h01, :],
                            e[:, off:], start=(i == 0), stop=(i == NSC - 1))
                for h01 in (0, 1):
                    h = 2 * c + h01
                    sums = epool.tile([P, 512], F32R, tag="sums", name="sums",
                                      bufs=2)
                    nc.vector.tensor_copy(sums[64:65, :],
                                          po[h01][DK:DK + 1, :])
                    # O_un += bv (x) sums  (folds the V bias through softmax)
                    nc.tensor.matmul(
                        po[h01][0:DK, :],
                        bv_tile[64:65, DK * h:DK * h + DK],
                        sums[64:65, :], start=False, stop=True,
                        skip_group_check=True)
                    nc.vector.reciprocal(sums[64:65, :], sums[64:65, :])
                    prb = ps_rb.tile([DK, 512], F32, tag="rb", name="rb")
                    nc.tensor.matmul(prb[:], prow_t[64:65, 0:DK],
                                     sums[64:65, :], start=True, stop=True)
                    # DVE can read only one PSUM operand: stage po first
                    o_un = epool.tile([DK, 512], F32, tag="o_un",
                                      name="o_un", bufs=2)
                    nc.vector.tensor_copy(o_un[:], po[h01][0:DK, :])
                    if h01 == 0:
                        nc.vector.tensor_mul(OT[c][0:DK, :], o_un[:], prb[:])
                    else:
                        ot = epool.tile([DK, 512], ot_dt, tag="ot",
                                        name="ot", bufs=1)
                        nc.vector.tensor_mul(ot[:], o_un[:], prb[:])
                        nc.sync.dma_start(OT[c][DK:P, :], ot[:])

        def out_proj_residual(w_tiles, bias_pn, OT_pairs, ps_acc, deq, pm,
                              dq_pool):
            for c2 in range(NKC):
                ps = ps_acc.tile([P, 512], F32, tag="proj0", name="proj0")
                for pc in range(NPC):
                    pair_mm(ps[:], w_tiles[pc], slice(c2 * P, (c2 + 1) * P),
                            OT_pairs[pc], slice(None), pc, NPC, pm)
                if pm is not None:
                    dq = dq_pool.tile([P, 512], F32, tag="odq", name="odq")
                    nc.scalar.activation(dq[:], ps[:], AF.Identity,
                                         bias=bias_pn[:, c2:c2 + 1],
                                         scale=deq)
                    nc.vector.tensor_add(xcur[c2][:], xcur[c2][:], dq[:])
                else:
                    nc.vector.scalar_tensor_tensor(
                        xcur[c2][:], ps[:], bias_pn[:, c2:c2 + 1], xcur[c2][:],
                        op0=ALU.add, op1=ALU.add)

        def attention_block(prefix, pn, KT, QT, vaug, mask_tiles, tail_pn,
                            use_smask, scope, causal=False):
            """Runs attention + output projection + residual."""
            pm = DR if cfg[f"{prefix}_o"] else None
            wo_dt = wo_sa_dt if prefix == "sa" else wo_ca_dt
            ot_dt = ot_sa_dt if prefix == "sa" else ot_ca_dt
            otp = scope.enter_context(
                tc.tile_pool(name=f"{pn}_otp", bufs=1))
            OT_pairs = [otp.tile([P, 2, OWN], ot_dt, tag=f"OT{c}",
                                 name=f"OT{c}") for c in range(NPC)]
            OT = [OT_pairs[c // 2][:, c % 2, :] for c in range(NKC)]
            # wo prefetch overlaps the attention phase
            wsp = scope.enter_context(tc.tile_pool(name=f"{pn}_wso", bufs=4))
            wo_tiles = load_w_pairs(wsp, f"{prefix}_wo", dt=wo_dt)
            with ExitStack() as att:
                ps_sc = att.enter_context(
                    tc.tile_pool(name=f"{pn}_psc", bufs=4, space="PSUM"))
                ps_av = att.enter_context(
                    tc.tile_pool(name=f"{pn}_pav", bufs=1, space="PSUM"))
                ps_rb = att.enter_context(
                    tc.tile_pool(name=f"{pn}_prb", bufs=2, space="PSUM"))
                epool = att.enter_context(
                    tc.tile_pool(name=f"{pn}_ep", bufs=5))
                attention(KT, QT, vaug, OT, mask_tiles, tail_pn,
                          smask if use_smask else None,
                          bias[f"{prefix}_bv"], (ps_sc, ps_av, ps_rb, epool),
                          g[f"att_{prefix}"], prb_row[prefix][:], ot_dt,
                          causal=causal, act_copy=(prefix == "sa"))
            with ExitStack() as oscope:
                pso = oscope.enter_context(
                    tc.tile_pool(name=f"{pn}_pso", bufs=4, space="PSUM"))
                dqp = oscope.enter_context(
                    tc.tile_pool(name=f"{pn}_dq", bufs=2))
                out_proj_residual(wo_tiles, bias[f"{prefix}_bo"], OT_pairs,
                                  pso, g[f"odeq_{prefix}"], pm, dqp)

        for _rep in range(repeat):
            # prefetch cross-attn memory + K weights; the DMAs drain during
            # the LN1/self-attention phases so the CA phase starts hot
            pref_cm = tc.tile_pool(name=f"ca_pref{_rep}", bufs=1)
            pref = pref_cm.__enter__()
            pmem_blocks = []
            for blk in range(2):
                mb = []
                for pc in range(NPC):
                    mt = pref.tile([P, 2, 512], mem_dt, tag=f"m{blk}_{pc}",
                                   name=f"m{blk}_{pc}")
                    nc.gpsimd.dma_start(
                        mt[:], dr["memT"][pc][:, :,
                                             blk * 512:(blk + 1) * 512])
                    mb.append(mt)
                pmem_blocks.append(mb)
            ca_wk_tiles = []
            for pc in range(NPC):
                wt = pref.tile([P, 2, D], wqkv_ca_dt, tag=f"pwk{pc}",
                               name=f"pwk{pc}")
                nc.gpsimd.dma_start(wt[:], dr["ca_wk"][pc])
                ca_wk_tiles.append(wt)
            cKT = [pref.tile([P, T], BF16, tag=f"cKT{c}", name=f"cKT{c}")[:]
                   for c in range(NKC)]
            cvaug = [pref.tile([P, H, DK + 1], BF16, tag=f"cV{i}",
                               name=f"cV{i}")[:] for i in range(NSC)]
            for i in range(NSC):
                nc.vector.memset(cvaug[i][:, :, DK], 1.0)

            # ==================================================================
            # phase 1+2: LN1, self-attention
            # ==================================================================
            with ExitStack() as sa:
                big = sa.enter_context(tc.tile_pool(name=f"sa_big{_rep}", bufs=1))
                QT = [big.tile([P, OWN], BF16, tag=f"QT{c}", name=f"QT{c}")[:]
                      for c in range(NKC)]
                KT = [big.tile([P, T], BF16, tag=f"KT{c}", name=f"KT{c}")[:]
                      for c in range(NKC)]
                vaug = [big.tile([P, H, DK + 1], BF16, tag=f"V{i}", name=f"V{i}")[:]
                        for i in range(NSC)]
                for i in range(NSC):
                    nc.vector.memset(vaug[i][:, :, DK], 1.0)

                with ExitStack() as ph:
                    h1p = ph.enter_context(
                        tc.tile_pool(name=f"h1{_rep}", bufs=1))
                    h1_pairs = [
                        [h1p.tile([P, 2, 512], h1_dt, tag=f"h1_{blk}_{pc}",
                                  name=f"h1_{blk}_{pc}")
                         for pc in range(NPC)]
                        for blk in range(2)]
                    h1_views = [
                        [h1_pairs[blk][kc // 2][:, kc % 2, :]
                         for kc in range(NKC)]
                        for blk in range(2)]
                    with ExitStack() as wscope:
                        # weight pool open during LN1 so the Q/K/V weight
                        # prefetch overlaps the LN chain
                        wsp = wscope.enter_context(
                            tc.tile_pool(name=f"sa_ws{_rep}", bufs=5))
                        psa = wscope.enter_context(
                            tc.tile_pool(name=f"ps_sap{_rep}", bufs=2,
                                         space="PSUM"))
                        with ExitStack() as lnscope:
                            xsp = lnscope.enter_context(
                                tc.tile_pool(name=f"xs{_rep}", bufs=3))
                            ps_ln = lnscope.enter_context(
                                tc.tile_pool(name=f"ps_ln1{_rep}", bufs=1,
                                             space="PSUM"))

                            def x_get(blk, kc):
                                xt = xsp.tile([P, 512], BF16, tag="xs",
                                              name="xs")
                                nc.gpsimd.dma_start(
                                    xt[:],
                                    dr["xT"][kc * P:(kc + 1) * P,
                                             blk * 512:(blk + 1) * 512])
                                return xt[:]

                            layer_norm(2, x_get, h1_views, lnrow["ln1"],
                                       lnp["ln1_b"], ps_ln)
                        # cross-attn K/V ride the same psum pool: their
                        # inputs are prefetched, so they fill LN1-wait gaps
                        proj_fm(wsp, "ca_wk", bias["ca_bk"], pmem_blocks,
                                cKT, psa, w_tiles=ca_wk_tiles,
                                pm=DR if cfg["ca_qkv"] else None)
                        proj_tm_vaug(wsp, "ca_wv", pmem_blocks, cvaug, psa,
                                     w_dt=wqkv_ca_dt,
                                     pm=DR if cfg["ca_qkv"] else None)
                        sa_pm = DR if cfg["sa_qkv"] else None
                        proj_fm(wsp, "sa_wq", bias["sa_bq"], [h1_pairs[0]],
                                QT, psa, w_dt=wqkv_sa_dt, pm=sa_pm)
                        proj_fm(wsp, "sa_wk", bias["sa_bk"], h1_pairs,
                                KT, psa, w_dt=wqkv_sa_dt, pm=sa_pm)
                        proj_tm_vaug(wsp, "sa_wv", h1_pairs, vaug, psa,
                                     w_dt=wqkv_sa_dt, pm=sa_pm)

                # residual base (own half of x): DMA bf16, upcast off-DVE
                with tc.tile_pool(name=f"xres{_rep}", bufs=2) as xrp:
                    for kc in range(NKC):
                        xs = xrp.tile([P, OWN], BF16, tag="xr", name="xr")
                        nc.sync.dma_start(
                            xs[:], dr["xT"][kc * P:(kc + 1) * P, 0:OWN])
                        nc.gpsimd.tensor_copy(xcur[kc][:], xs[:])

                with ExitStack() as mscope:
                    mask_tiles = []
                    if mask_mode != "causal":
                        mp = mscope.enter_context(
                            tc.tile_pool(name=f"maskp{_rep}", bufs=1))
                        nmt = 4 if mask_mode == "split" else NSC
                        for i in range(nmt):
                            mt = mp.tile([P, OWN], F32, tag=f"mask{i}",
                                         name=f"mask{i}")
                            nc.sync.dma_start(
                                mt[:], dr["maskT"][i * P:(i + 1) * P, :])
                            mask_tiles.append(mt[:])
                    attention_block(
                        "sa", f"sa{_rep}", KT, QT, vaug, mask_tiles,
                        tailb if mask_mode in ("split", "causal") else None,
                        False, mscope, causal=(mask_mode == "causal"))

            # ==================================================================
            # phase 3: cross attention
            # ==================================================================
            with ExitStack() as ca:
                big = ca.enter_context(tc.tile_pool(name=f"ca_big{_rep}", bufs=1))
                QT = [big.tile([P, OWN], BF16, tag=f"cQT{c}", name=f"cQT{c}")[:]
                      for c in range(NKC)]
                KT = cKT
                vaug = cvaug

                with ExitStack() as mm_scope:
                    wsp = mm_scope.enter_context(
                        tc.tile_pool(name=f"ca_wkv{_rep}", bufs=5))
                    pca = mm_scope.enter_context(
                        tc.tile_pool(name=f"ps_ckv{_rep}", bufs=2,
                                     space="PSUM"))

                    with ExitStack() as ph:
                        h2p = ph.enter_context(
                            tc.tile_pool(name=f"h2{_rep}", bufs=1))
                        h2_pairs = [h2p.tile([P, 2, OWN], h2_dt,
                                             tag=f"h2_{pc}", name=f"h2_{pc}")
                                    for pc in range(NPC)]
                        h2_views = [h2_pairs[kc // 2][:, kc % 2, :]
                                    for kc in range(NKC)]
                        with tc.tile_pool(name=f"ps_ln2{_rep}", bufs=1,
                                          space="PSUM") as psl:
                            layer_norm(1, lambda blk, kc: xcur[kc][:],
                                       [h2_views], lnrow["ln2"],
                                       lnp["ln2_b"], psl)
                        proj_fm(wsp, "ca_wq", bias["ca_bq"], [h2_pairs], QT,
                                pca, w_dt=wqkv_ca_dt,
                                pm=DR if cfg["ca_qkv"] else None)

                with ExitStack() as ascope:
                    attention_block("ca", f"ca{_rep}", KT, QT, vaug, None,
                                    None, True, ascope)

            pref_cm.__exit__(None, None, None)

            # ==================================================================
            # phase 4: FFN
            # ==================================================================
            with ExitStack() as ff:
                ap_pool = ff.enter_context(tc.tile_pool(name=f"aT{_rep}", bufs=1))
                w2p = ff.enter_context(tc.tile_pool(name=f"w2p{_rep}", bufs=1))
                aT_pairs = [ap_pool.tile([P, 2, OWN], aT_dt, tag=f"aT{i}",
                                         name=f"aT{i}") for i in range(NFP)]
                aT = [aT_pairs[i // 2][:, i % 2, :] for i in range(NFC)]
                ff1_pm = DR if cfg["ff1"] else None
                ff2_pm = DR if cfg["ff2"] else None
                with ExitStack() as ph:
                    h3p = ph.enter_context(
                        tc.tile_pool(name=f"h3{_rep}", bufs=1))
                    h3_pairs = [h3p.tile([P, 2, OWN], h3_dt, tag=f"h3_{pc}",
                                         name=f"h3_{pc}")
                                for pc in range(NPC)]
                    h3_views = [h3_pairs[kc // 2][:, kc % 2, :]
                                for kc in range(NKC)]
                    w2_tiles = []
                    with ExitStack() as wscope:
                        wsp = wscope.enter_context(
                            tc.tile_pool(name=f"ff_ws{_rep}",
                                         bufs=8 if cfg["ff1"] else 4))
                        ps_f1 = wscope.enter_context(
                            tc.tile_pool(name=f"ps_ff1{_rep}",
                                         bufs=2 if cfg["ff2"] else 4,
                                         space="PSUM"))
                        with tc.tile_pool(name=f"ps_ln3{_rep}", bufs=1,
                                          space="PSUM") as psl:
                            layer_norm(1, lambda blk, kc: xcur[kc][:],
                                       [h3_views], lnrow["ln3"],
                                       lnp["ln3_b"], psl)
                        for cg in range(4):
                            w1t = []
                            for pc in range(NPC):
                                wt = wsp.tile([P, 2, 1024], w1_dt, tag="w",
                                              name="w")
                                nc.sync.dma_start(
                                    wt[:], dr["w1"][pc][:, :, cg * 1024:
                                                        (cg + 1) * 1024])
                                w1t.append(wt)
                            # stream the second-layer weights in during the
                            # first-layer compute so FFN2 starts hot (fp8
                            # only: f32r weights don't fit resident)
                            if cfg["ff2"]:
                                for pc16 in range(cg * 4, cg * 4 + 4):
                                    wt = w2p.tile([P, 2, D], w2_dt,
                                                  tag=f"w2_{pc16}",
                                                  name=f"w2_{pc16}")
                                    nc.sync.dma_start(wt[:], dr["w2"][pc16])
                                    w2_tiles.append(wt)
                            for cc in range(NKC):
                                cidx = cg * 8 + cc
                                ps = ps_f1.tile([P, 512], F32, tag="proj0",
                                                name="proj0")
                                for pc in range(NPC):
                                    pair_mm(ps[:], w1t[pc],
                                            slice(cc * P, (cc + 1) * P),
                                            h3_pairs[pc], slice(None),
                                            pc, NPC, ff1_pm)
                                nc.scalar.activation(
                                    aT[cidx], ps[:], AF.Relu,
                                    bias=bias["b1"][:, cidx:cidx + 1],
                                    scale=g["relu_s"])

                with ExitStack() as yscope:
                    ps_y = yscope.enter_context(
                        tc.tile_pool(name=f"ps_y{_rep}", bufs=1, space="PSUM"))
                    if cfg["ff2"]:
                        # two groups of 4 output chunks: 4 PSUM banks each,
                        # so FFN2 fits alongside FFN1's accumulators
                        for grp in range(2):
                            yps = [ps_y.tile([P, 512], F32, tag=f"y{j}",
                                             name=f"y{j}") for j in range(4)]
                            for pc16 in range(NFP):
                                for j in range(4):
                                    c2 = grp * 4 + j
                                    pair_mm(yps[j][:], w2_tiles[pc16],
                                            slice(c2 * P, (c2 + 1) * P),
                                            aT_pairs[pc16], slice(None),
                                            pc16, NFP, ff2_pm)
                            for j in range(4):
                                c2 = grp * 4 + j
                                # b2 is added on the host after gather
                                nc.vector.scalar_tensor_tensor(
                                    xcur[c2][:], yps[j][:], g["ydeq"],
                                    xcur[c2][:], op0=ALU.mult, op1=ALU.add)
                    else:
                        yps = [ps_y.tile([P, 512], F32, tag=f"y{c2}",
                                         name=f"y{c2}")
                               for c2 in range(NKC)]
                        for pc16 in range(NFP):
                            wt = w2p.tile([P, 2, D], w2_dt, tag="w", name="w",
                                          bufs=4)
                            nc.sync.dma_start(wt[:], dr["w2"][pc16])
                            for c2 in range(NKC):
                                pair_mm(yps[c2][:], wt,
                                        slice(c2 * P, (c2 + 1) * P),
                                        aT_pairs[pc16], slice(None),
                                        pc16, NFP, ff2_pm)
                        for c2 in range(NKC):
                            nc.vector.scalar_tensor_tensor(
                                xcur[c2][:], yps[c2][:],
                                bias["b2"][:, c2:c2 + 1], xcur[c2][:],
                                op0=ALU.add, op1=ALU.add)

            for c2 in range(NKC):
                nc.sync.dma_start(outT[c2 * P:(c2 + 1) * P, :], xcur[c2][:])

    nc.finalize()
    return nc


# ---------------------------------------------------------------------------
# host side
# ---------------------------------------------------------------------------

def _pairize(wT, dt_np, scale):
    """[D_in, O] -> [D_in/256, P, 2, O] pair-chunk layout for DoubleRow."""
    di, do = wT.shape
    w = wT.astype(np.float32) * scale
    if dt_np is NPF8:
        w = np.clip(w, -240.0, 240.0)
    return np.ascontiguousarray(
        w.reshape(di // 256, 2, P, do).transpose(0, 2, 1, 3)).astype(dt_np)


def host_prep(inputs, cfg=None):
    """Returns (per_core_maps, consts). consts holds everything that is
    identical across cores and calls (weights, biases, LN params, fixed
    masks) for embedding into the NEFF; maps hold the per-core runtime
    inputs (xT, memT, smdyn, and maskT for non-causal fallback)."""
    cfg = dict(CFG if cfg is None else cfg)
    g = _gains(cfg)
    f32 = np.float32
    x = np.asarray(inputs["x"], f32)
    mem = np.asarray(inputs["memory"], f32)
    tgt = np.asarray(inputs["tgt_mask"])
    src = np.asarray(inputs["src_mask"])

    add_tgt = (tgt.astype(f32) - 1.0) * MASKV    # [B, T, T]: 0 or -2^62
    add_src = (src.astype(f32) - 1.0) * 1e9      # [B, T] (post-scale bias)

    w_np = {True: NPF8, False: NPBF}
    consts = {"ones_r": np.ones((P, P), f32)}
    kk = np.arange(P, dtype=np.int64)
    consts["trimask"] = np.where(kk[:, None] <= kk[None, :],
                                 np.float32(0.0), np.float32(-MASKV))
    for pre in ("sa", "ca"):
        qkv_on = cfg[f"{pre}_qkv"]
        o_on = cfg[f"{pre}_o"]
        qdt = w_np[qkv_on]
        for nm in ("wq", "wk", "wv"):
            consts[f"{pre}_{nm}"] = _pairize(
                np.asarray(inputs[f"{pre}_{nm}"], f32).T, qdt,
                SW if qkv_on else 1.0)
        consts[f"{pre}_wo"] = _pairize(
            np.asarray(inputs[f"{pre}_wo"], f32).T, w_np[o_on],
            SW if o_on else 1.0)
        consts[f"{pre}_bv"] = (np.asarray(inputs[f"{pre}_bv"], f32)
                               * g[f"v_{pre}"])
    sm_cols = []
    for pre in ("sa", "ca"):
        qg = g["qk_sa"] if pre == "sa" else g["q_ca"]
        kg = g["qk_sa"] if pre == "sa" else g["k_ca"]
        sm_cols.append((np.asarray(inputs[f"{pre}_bq"], f32)
                        * qg).reshape(NKC, P).T)
        sm_cols.append((np.asarray(inputs[f"{pre}_bk"], f32)
                        * kg).reshape(NKC, P).T)
        sm_cols.append(np.asarray(inputs[f"{pre}_bo"],
                                  f32).reshape(NKC, P).T)
    consts["lnwrows"] = np.concatenate(
        [np.asarray(inputs[f"{ln}_w"], f32) * g[h]
         for ln, h in (("ln1", "h1"), ("ln2", "h2"), ("ln3", "h3"))]
    ).reshape(1, 3 * D)
    ff_np = f32 if cfg.get("ff_f32r") else NPBF
    consts["w1"] = _pairize(np.asarray(inputs["ff_w1"], f32).T,
                            NPF8 if cfg["ff1"] else ff_np,
                            SW if cfg["ff1"] else 1.0)
    consts["w2"] = _pairize(np.asarray(inputs["ff_w2"], f32).T,
                            NPF8 if cfg["ff2"] else ff_np,
                            SW if cfg["ff2"] else 1.0)
    # order must match sa_bq/sa_bk/sa_bo, ca_bq/ca_bk/ca_bo above
    sm_fixed = [sm_cols[0], sm_cols[1], sm_cols[2],
                sm_cols[3], sm_cols[4], sm_cols[5],
                (np.asarray(inputs["ff_b1"], f32)
                 * g["aT"]).reshape(NFC, P).T,
                np.asarray(inputs["ff_b2"], f32).reshape(NKC, P).T]
    for ln, h in (("ln1", "h1"), ("ln2", "h2"), ("ln3", "h3")):
        sm_fixed.append((np.asarray(inputs[f"{ln}_b"], f32)
                         * g[h]).reshape(NKC, P).T)
    consts["smalls"] = np.ascontiguousarray(
        np.concatenate(sm_fixed, axis=1))

    maps = []
    for c in range(8):
        b, half = c // 2, c % 2
        q0 = half * OWN
        order = np.concatenate(
            [np.arange(q0, q0 + OWN), np.r_[0:q0, q0 + OWN:T]]).astype(
                np.int64)
        m = {}
        m["xT"] = np.ascontiguousarray(x[b][order].T).astype(NPBF)
        memT = np.ascontiguousarray(mem[b].T) * g["mem"]
        if cfg["ca_qkv"]:
            memT = np.clip(memT, -240.0, 240.0)
        m["memT"] = np.ascontiguousarray(
            memT.reshape(NPC, 2, P, T).transpose(0, 2, 1, 3)).astype(
                w_np[cfg["ca_qkv"]])
        mt = np.ascontiguousarray(add_tgt[b][q0:q0 + OWN][:, order].T)
        m["maskT"] = mt
        smask_c = add_src[b].reshape(NSC, P).T
        tailb_c = np.where(mt[OWN:, 0] < 0, np.float32(-1e9),
                           np.float32(0.0)).reshape(4, P).T
        m["smdyn"] = np.ascontiguousarray(
            np.concatenate([smask_c, tailb_c], axis=1))
        maps.append(m)
    return maps, consts


def _tail_rows_constant(maps):
    """True when every core's mask s-chunks 4..7 are constant per s-row, so
    they can be applied as a per-partition exp bias instead of tensor adds."""
    for m in maps:
        tail = m["maskT"][OWN:, :]
        if not np.all(tail == tail[:, :1]):
            return False
    return True


def _mask_mode(maps):
    """"causal" when the own-key block of every core's mask is exactly
    lower-triangular (key s <= query q visible) — the program then applies a
    hardcoded tri block on the diagonal and skips fully-masked regions."""
    if not _tail_rows_constant(maps):
        return "full"
    ss = np.arange(OWN)
    tril = ss[:, None] <= ss[None, :]
    for m in maps:
        if not np.array_equal(m["maskT"][:OWN] < 0, ~tril):
            return "split"
    return "causal"


def gather(results, inputs=None, cfg=None):
    cfg = dict(CFG if cfg is None else cfg)
    out = np.zeros((B, T, D), np.float32)
    for c in range(8):
        b, half = c // 2, c % 2
        out[b, half * OWN:(half + 1) * OWN, :] = results[c]["outT"].T
    if cfg["ff2"] and inputs is not None:
        out += np.asarray(inputs["ff_b2"], np.float32)
    return out


_NC_CACHE = {}


def _digest(consts):
    import hashlib
    h = hashlib.blake2b(digest_size=16)
    for k in sorted(consts):
        a = np.ascontiguousarray(consts[k])
        h.update(k.encode())
        h.update(str(a.shape).encode())
        h.update(str(a.dtype).encode())
        h.update(a.tobytes())
    return h.hexdigest()


def kernel(**inputs):
    in_maps, consts = host_prep(inputs)
    mode = _mask_mode(in_maps)
    key = (mode, _digest(consts))
    if key not in _NC_CACHE:
        _NC_CACHE[key] = build_program(mask_mode=mode, consts=consts)
    nc = _NC_CACHE[key]
    res = run_bass_kernel_spmd(nc, in_maps, list(range(8)))
    return gather(res.results, inputs)


if __name__ == "__main__":
    import reference as ref_mod
    inputs = {k: np.asarray(v) for k, v in ref_mod.setup_inputs().items()}
    expected = np.asarray(ref_mod.reference(**ref_mod.setup_inputs()))
    actual = kernel(**inputs)
    err = np.abs(actual - expected).max()
    rel = err / np.abs(expected).max()
    print("max abs err:", err, "rel:", rel)
